# revision 34
# baseline (speedup 1.0000x reference)
"""Trainium2 Bass kernel for nn_CoNe_35974646071945 (retrieval_knn).

Strategy: K-shard the 65536-entry queue across 8 NeuronCores. Host pre-casts
all inputs to fp8 e4m3 so the device reads a quarter of the f32 bytes and
every matmul runs in fp8 DoubleRow mode. Each core:
  simqT[j, b] = (queue8_shard)^T @ norm_q8^T    (fp8 DR matmul -> bf16 out)
  simkT[j, b] = (queue8_shard)^T @ k_feat8^T    (fp8 DR matmul)
  ET[j, b]    = exp(simkT / T_DC)               (fp8, SBUF-resident)
  P[b, cls+]  = ET^T @ [qlp8_shard^T * S | 1]   (fp8 DR matmuls, fp32 PSUM
                                                 accumulated over the shard;
                                                 ones column gives the
                                                 softmax partition Z)
Host: the device sims only PRESELECT top-M=2*knn candidates per row; the
host recomputes exact f32 sims for those M and does the exact top-200 /
softmax on them (validated: zero membership misses, supin exact to 1e-7).
P partials are summed over cores; softmax/KL on tiny arrays.

Layout notes: queue loads are chunked along j so the first matmuls start
~2us in; simq writes are batched 8 j-tiles per DMA; qlp streams in 16
half-MB groups, triggers rate-matched (last 4 phase-1 chunks + 6-group
lead inside phase 2's first bt pass) so they never starve phase-1
write-backs nor phase 2. All PSUM matmul outputs are single-bank (the
512/496 split of the 1008-wide dc accumulation is load-bearing: a chunk
straddling a 2KB PSUM bank boundary corrupts the accumulation).
"""
import sys
sys.path.insert(0, '/opt/trn_rl_repo')
sys.path.insert(0, '/root/.axon_site/_ro/trn_rl_repo')

import numpy as np
import ml_dtypes
from contextlib import ExitStack

from concourse import bass, tile, mybir
from concourse.bass_utils import run_bass_kernel_spmd
from concourse.vector_clock import ScopedClock, VectorClock

F32 = mybir.dt.float32
BF16 = mybir.dt.bfloat16
F8 = mybir.dt.float8e4
Alu = mybir.AluOpType
Act = mybir.ActivationFunctionType
DR = mybir.MatmulPerfMode.DoubleRow

NP_BF16 = ml_dtypes.bfloat16
NP_F8 = ml_dtypes.float8_e4m3

N_CORES = 8
B, D, K, C = 512, 256, 65536, 1000
KS = K // N_CORES            # 8192 queue columns per core
T_SUP, T_DC, LS = 0.07, 0.1, 0.1
EPS = 1e-8
NJT = KS // 128              # 64 j-tiles per core
NST = NJT // 2               # 32 super-tiles (256 j) for DoubleRow
NG = NST // 2                # 16 qlp DMA groups (512 j each)
C1 = 1008                    # 1000 classes + ones col (idx 1000) + pad
NCH = 8                      # simq output chunks (8 j-tiles each)
NLC = 4                      # queue load chunks (2048 j-cols each)
LCW = KS // NLC


class CompatTileContext(tile.TileContext):
    """This walrus build encodes at most ONE sync wait per instruction.
    Split Tile's multi-wait instructions and its tail drain."""

    def _commit_instruction(self, inst, lazy_reg_writes=True):
        si = inst.sync_info
        if (
            si is not None
            and si.on_wait
            and len(si.on_wait) > 1
            and inst.engine != mybir.EngineType.Unassigned
        ):
            import bass_rust
            waits = list(si.on_wait)
            for w in waits[:-1]:
                nop = mybir.InstNoOp(
                    name=f"I-{self.nc.next_id()}", ins=[], outs=[]
                )
                nop.engine = inst.engine
                nop.sync_info = bass_rust.SyncInfo(on_wait=[w], on_update=[])
                super()._commit_instruction(nop, lazy_reg_writes=False)
            si.on_wait = [waits[-1]]
            inst.sync_info = si
        super()._commit_instruction(inst, lazy_reg_writes=lazy_reg_writes)

    def _drain_and_barrier(self, tick_clock, wait_clock):
        gclock = tick_clock.global_clock
        n = len(gclock)
        for i in range(n):
            if gclock[i] == 0:
                continue
            vec = [0] * n
            vec[i] = gclock[i]
            nop_inst = self.nc.sync.nop(nofuse=True, hint=f"tail_wait_p{i}")
            wait_clock.add_sem_waits(
                nop_inst.ins, ScopedClock({None: VectorClock(vec)})
            )
        self.nc.sync.drain()
        self.nc.all_engine_barrier()
        assert self.sems is not None
        popped = self.nc._tile_sem_poison_stack.pop()
        assert popped is self._sem_poison
        self.nc.clear_and_free_semaphores(list(self.sems.allocated().values()))
        self.nc.all_engine_barrier()


_CACHED = {}


def _build():
    if 'nc' in _CACHED:
        return _CACHED['nc']
    nc = bass.Bass(num_devices=N_CORES)
    # fp8 inputs come pre-arranged host-side as [p, d_half, free] so each
    # tensor/chunk needs a single DMA trigger
    qT8_in = nc.declare_dram_parameter("qT8", [128, 2, B], F8, isOutput=False)
    kT8_in = nc.declare_dram_parameter("kT8", [128, 2, B], F8, isOutput=False)
    qsh8_in = nc.declare_dram_parameter("qsh8", [128, 2, KS], F8, isOutput=False)
    qlp8_in = nc.declare_dram_parameter(
        "qlp8", [NG * 128, 4, C1], F8, isOutput=False)
    # simq[c*128+p, sub, b] = sim[j = c*1024 + sub*128 + p, b]
    simq_out = nc.declare_dram_parameter(
        "simq", [NCH * 128, NJT // NCH, B], BF16, isOutput=True)
    p_out = nc.declare_dram_parameter("pout", [B, C1], F32, isOutput=True)

    with ExitStack() as ctx:
        tc = ctx.enter_context(CompatTileContext(nc))
        pool = ctx.enter_context(tc.tile_pool(name="main", bufs=1))
        qstg = ctx.enter_context(tc.tile_pool(name="qstg", bufs=1))
        sq = ctx.enter_context(tc.tile_pool(name="sq", bufs=4))
        pc = ctx.enter_context(tc.tile_pool(name="pc", bufs=4))

        # moving operands first: norm_q^T / k_feat^T fp8, [d, 2, b]
        qT8 = pool.tile([128, 2, B], F8, name="qT8")
        kT8 = pool.tile([128, 2, B], F8, name="kT8")
        nc.sync.dma_start(kT8[:], kT8_in[:, :, :])
        nc.sync.dma_start(qT8[:], qT8_in[:, :, :])

        # stationary queue shard, chunked along j (small first chunk so the
        # first matmuls start as early as possible)
        q8 = pool.tile([128, 2, KS], F8, name="q8")
        bounds = [0, 512, 2560, 4608, 6656, KS]
        for lo, hi in zip(bounds, bounds[1:]):
            nc.sync.dma_start(q8[:, :, lo:hi], qsh8_in[:, :, lo:hi])

        # qlp tiles: DMAs are issued rate-matched inside the phase-1 loop
        # (two groups per simq chunk) so they neither starve phase-1's
        # write-backs nor leave phase 2 waiting
        qls = [qstg.tile([128, 4, C1], F8, name=f"ql{g}") for g in range(NG)]

        # ET (exp(simk/T_DC)) fp8, [128, NJT, 512]
        et = pool.tile([128, NJT, B], F8, name="et")

        # P accumulators for b-chunks 0/1 live through phase 1 so their
        # matmuls can interleave with the elementwise-bound phase-1 stream.
        # [128, 1024] f32 = exactly 2 banks per tile: each matmul chunk
        # ([:, :512] / [:, 512:C1]) stays within one bank.
        ps2a = ctx.enter_context(
            tc.tile_pool(name="ps2a", bufs=1, space="PSUM"))
        pacc01 = [ps2a.tile([128, 1024], F32, name=f"pacc{bt}")
                  for bt in range(2)]

        def p_quarter(pacc, bt, g, sl):
            """2 DoubleRow MMs accumulating qlp half-group (g, sl) into pacc."""
            ql = qls[g]
            s = 2 * g + sl
            st = (s == 0)
            sp = (s == NST - 1)
            lhs = et[:, 2 * s:2 * s + 2, bt * 128:(bt + 1) * 128]
            nc.tensor.matmul(pacc[:, :512], lhs,
                             ql[:, 2 * sl:2 * sl + 2, :512],
                             start=st, stop=sp, perf_mode=DR)
            nc.tensor.matmul(pacc[:, 512:C1], lhs,
                             ql[:, 2 * sl:2 * sl + 2, 512:],
                             start=st, stop=sp, perf_mode=DR)

        # quarter-slot schedule for the interleaved bt0/bt1 accumulation:
        # slot idx -> (g, part); parts 0,1 = bt0 sl0/sl1, parts 2,3 = bt1
        def p_slot(idx):
            g, part = idx // 4, idx % 4
            p_quarter(pacc01[part // 2], part // 2, g, part % 2)

        def p_copyout(pacc, bt):
            # split across Vector+Scalar (different PSUM banks) in parallel
            pcp = pc.tile([128, C1], F32, name="pcp", tag="pcp")
            nc.vector.tensor_copy(pcp[:, :512], pacc[:, :512])
            nc.scalar.activation(pcp[:, 512:C1], pacc[:, 512:C1],
                                 Act.Copy, scale=1.0)
            nc.sync.dma_start(p_out[bt * 128:(bt + 1) * 128, :], pcp[:])

        # early qlp prefetch for the interleaved groups
        for g in (0, 1):
            nc.sync.dma_start(qls[g][:], qlp8_in[g * 128:(g + 1) * 128, :, :])

        # HAM warmup: dummy matmuls on a zeroed tile run during the startup
        # DMA window (no data deps), so the PE clock is at 2.4 GHz when the
        # real stream starts instead of paying ~3.4us of 1.2 GHz cold time.
        with ExitStack() as wu:
            wup = wu.enter_context(
                tc.tile_pool(name="warm", bufs=1, space="PSUM"))
            zmm = pool.tile([128, B], F8, name="zmm")
            nc.gpsimd.memset(zmm[:], 0)
            wps = wup.tile([128, B], F32, name="wps")
            for _ in range(10):
                nc.tensor.matmul(wps[:], zmm[:, :128], zmm[:],
                                 start=True, stop=True)

        # phase 1 (merged): per j-tile simk/simq fp8 DR matmuls + exp/cast,
        # with phase-2 group MMs for b-chunks 0/1 interleaved into the PE
        # stream (PE is otherwise ~50% idle here, gated by Scalar/Vector).
        with ExitStack() as ph1:
            psk = ph1.enter_context(
                tc.tile_pool(name="psk", bufs=2, space="PSUM"))
            psq = ph1.enter_context(
                tc.tile_pool(name="psq", bufs=2, space="PSUM"))
            for ch in range(NCH):
                sqt = sq.tile([128, NJT // NCH, B], BF16, name="sqt", tag="sqt")
                for sub in range(NJT // NCH):
                    t = ch * (NJT // NCH) + sub
                    jl = t * 128
                    pk = psk.tile([128, B], F32, name="pk", tag="pk")
                    pq = psq.tile([128, B], F32, name="pq", tag="pq")
                    nc.tensor.matmul(pk[:], q8[:, :, jl:jl + 128],
                                     kT8[:, :, :],
                                     start=True, stop=True, perf_mode=DR)
                    nc.tensor.matmul(pq[:], q8[:, :, jl:jl + 128],
                                     qT8[:, :, :],
                                     start=True, stop=True, perf_mode=DR)
                    nc.scalar.activation(et[:, t:t + 1, :], pk[:],
                                         Act.Exp, scale=1.0 / T_DC)
                    nc.vector.tensor_copy(sqt[:, sub:sub + 1, :], pq[:])
                    if t % 4 == 2 and t // 4 + 2 < NG:
                        g = t // 4 + 2
                        nc.sync.dma_start(
                            qls[g][:], qlp8_in[g * 128:(g + 1) * 128, :, :])
                    if t >= 11 and (t - 11) % 4 == 0:
                        for part in range(4):
                            p_slot(4 * ((t - 11) // 4) + part)
                nc.sync.dma_start(
                    simq_out[ch * 128:(ch + 1) * 128, :, :], sqt[:])
            for idx in range(4 * (NG - 2), 4 * NG):
                p_slot(idx)

        # phase 2 remainder: b-chunks 2/3 (psk/psq banks now free);
        # pacc0/1 copy-out overlaps the bt=2 matmul stream
        with ExitStack() as ph2:
            ps2b = ph2.enter_context(
                tc.tile_pool(name="ps2b", bufs=1, space="PSUM"))
            pacc23 = [ps2b.tile([128, 1024], F32, name=f"pacc{bt}")
                      for bt in (2, 3)]
            for g in range(NG):
                p_quarter(pacc23[0], 2, g, 0)
                p_quarter(pacc23[0], 2, g, 1)
                if g == 0:
                    p_copyout(pacc01[0], 0)
                    p_copyout(pacc01[1], 1)
            p_copyout(pacc23[0], 2)
            # bt3: two single-bank passes so the first bank's copy-out and
            # DMA overlap the second bank's matmul stream
            pcp3 = pc.tile([128, C1], F32, name="pcp", tag="pcp")
            for g in range(NG):
                for sl in range(2):
                    s = 2 * g + sl
                    lhs = et[:, 2 * s:2 * s + 2, 3 * 128:4 * 128]
                    nc.tensor.matmul(
                        pacc23[1][:, :512], lhs,
                        qls[g][:, 2 * sl:2 * sl + 2, :512],
                        start=(s == 0), stop=(s == NST - 1), perf_mode=DR)
            nc.vector.tensor_copy(pcp3[:, :512], pacc23[1][:, :512])
            nc.sync.dma_start(p_out[3 * 128:4 * 128, :512], pcp3[:, :512])
            for g in range(NG):
                for sl in range(2):
                    s = 2 * g + sl
                    lhs = et[:, 2 * s:2 * s + 2, 3 * 128:4 * 128]
                    nc.tensor.matmul(
                        pacc23[1][:, 512:C1], lhs,
                        qls[g][:, 2 * sl:2 * sl + 2, 512:],
                        start=(s == 0), stop=(s == NST - 1), perf_mode=DR)
            nc.scalar.activation(pcp3[:, 512:C1], pacc23[1][:, 512:C1],
                                 Act.Copy, scale=1.0)
            nc.sync.dma_start(p_out[3 * 128:4 * 128, 512:], pcp3[:, 512:C1])

    _CACHED['nc'] = nc
    return nc


def _prep_inputs(norm_q, k_feat, queue, qlp):
    """Host-side cast + layout. Returns (in_maps, S)."""
    mx = float(qlp.max())
    S = float(2.0 ** np.floor(np.log2(200.0 / max(mx, 1e-20))))
    def dev3(a):  # [D, N] f32 -> [128, 2, N] fp8, dev[p, o, n] = a[o*128+p, n]
        return np.ascontiguousarray(
            a.astype(NP_F8).reshape(2, 128, -1).transpose(1, 0, 2))
    qT8 = dev3(np.ascontiguousarray(norm_q.T))
    kT8 = dev3(np.ascontiguousarray(k_feat.T))
    in_maps = []
    for c in range(N_CORES):
        sh = slice(c * KS, (c + 1) * KS)
        # qlp shard -> [KS, C1] fp8 with ones col + pad, then grouped
        # DoubleRow interleave [NG*128, 4, C1]
        aug = np.zeros((KS, C1), np.float32)
        aug[:, :C] = qlp[:, sh].T * S
        aug[:, C] = 1.0
        aug8 = aug.astype(NP_F8)
        qlp8 = np.ascontiguousarray(
            aug8.reshape(NG, 2, 2, 128, C1).transpose(0, 3, 1, 2, 4)
        ).reshape(NG * 128, 4, C1)
        in_maps.append({
            "qT8": qT8, "kT8": kT8,
            "qsh8": dev3(np.ascontiguousarray(queue[:, sh])),
            "qlp8": qlp8,
        })
    return in_maps, S


def kernel(norm_q, q_logits, k_feat, logits_k, queue, queue_label_prob,
           queue_label, target, knn_k):
    norm_q = np.asarray(norm_q, np.float32)
    q_logits = np.asarray(q_logits, np.float32)
    k_feat = np.asarray(k_feat, np.float32)
    queue = np.asarray(queue, np.float32)
    qlp = np.asarray(queue_label_prob, np.float32)
    queue_label = np.asarray(queue_label)
    target = np.asarray(target)
    kk = int(knn_k)

    nc = _build()
    in_maps, S = _prep_inputs(norm_q, k_feat, queue, qlp)
    res = run_bass_kernel_spmd(nc, in_maps, list(range(N_CORES)))

    # simq[c, p, sub, b] -> sim rows j = c*1024 + sub*128 + p
    blocks = []
    for c in range(N_CORES):
        A = res.results[c]["simq"].astype(np.float32)
        A = A.reshape(NCH, 128, NJT // NCH, B).transpose(0, 2, 1, 3)
        blocks.append(A.reshape(KS, B).T)
    sim = np.concatenate(blocks, axis=1)
    P = np.zeros((B, C1), np.float64)
    for c in range(N_CORES):
        P += res.results[c]["pout"].astype(np.float64)

    # ---- supcon: coarse top-M from device fp8 sims, exact f32 refine ----
    M = min(kk + 312, K)   # validated: zero top-kk misses at kk=200, M=512
    cidx = np.argpartition(-sim, M - 1, axis=1)[:, :M]
    gath = queue.T[cidx]                       # [B, M, D]
    ref = np.einsum('bmd,bd->bm', gath, norm_q)
    sel = np.argpartition(-ref, kk - 1, axis=1)[:, :kk]
    idx = np.take_along_axis(cidx, sel, axis=1)
    sim_knn = np.take_along_axis(ref, sel, axis=1)
    w = np.exp((sim_knn - sim_knn.max(axis=1, keepdims=True)) / T_SUP)
    w /= w.sum(axis=1, keepdims=True)
    pos = (target[:, None] == queue_label[idx])
    gt = (w * pos).sum(axis=1)
    m = gt > EPS
    supin_loss = np.where(m, -np.log(np.where(m, gt, 1.0)), 0.0).sum() / B

    # ---- fc loss ----
    x = q_logits.astype(np.float64)
    lse = np.log(np.exp(x - x.max(1, keepdims=True)).sum(1)) + x.max(1)
    log_q = x - lse[:, None]
    q_mask = (x.min(1) - lse) > np.log(EPS)
    onehot = np.full((B, C), LS / (C - 1))
    onehot[np.arange(B), target] = 1.0 - LS
    fc_loss = -((onehot * log_q).sum(1) * q_mask).sum() / B

    # ---- dc loss ----
    Z = P[:, C] * S
    dc_t = P[:, :C] / Z[:, None]
    dc_pos = dc_t > 0
    kl = np.where(dc_pos,
                  dc_t * (np.log(np.where(dc_pos, dc_t, 1.0)) - log_q), 0.0)
    dc_loss = (kl.sum(1) * q_mask).sum() / B

    return (np.float32(supin_loss), np.float32(fc_loss), np.float32(dc_loss))
